# revision 14
# baseline (speedup 1.0000x reference)
"""Trainium2 Bass kernel for nn_CaptioningRNN (attention LSTM over T=64).

Data-parallel over the batch: N=256 samples split across 8 NeuronCores
(32 samples/core), weights replicated, no collectives.

Per-core algorithm (v3):
  - No xproj prepass: x @ Wx is accumulated directly into each step's gate
    strips on the TensorEngine (x-chunk stationary [128,32], Wx moving,
    4-way column tiling), emitted one step ahead so it executes during the
    previous step's vector/scalar tail. This removes the xps DRAM scratch
    round-trip entirely.
  - Strips are laid out [32*qh + n, (j, p)]: the 4 PE column-tile quadrants
    map to 128-column blocks (qh) of the hidden dim, NOT the gate index, so
    i/f/o/g for one (n, h') live on one partition and the LSTM cell math
    runs directly in strip space (no [128,512] transposes).
  - P[n,k,:] = A[n,:,k] @ Wattn precomputed once; the bias b is folded into
    P (softmax weights sum to 1), so gates = xWx + hWh + sum_k w_k P_k
    includes +b exactly.
  - scores via 4 column-tiled accumulation chains into one [128,512] PSUM,
    diag-masked and reduced on DVE, block-summed with a tiny PE matmul.
  - softmax exp computed as sigmoid(s-m)/(1-sigmoid(s-m)) so the scalar
    engine never swaps activation tables (Exp <-> Sigmoid/Tanh reload costs
    ~2.6us/step otherwise).
  - h is produced per 512-block as [s, p] f32, DMA'd straight to the output,
    and PE-transposed ([128,128]) to hT bf16 for the next step's stationary.
  - x streamed in quarters (16 steps each) to fit SBUF next to Wx+Wh+P.
"""

import numpy as np
import ml_dtypes

import concourse.bacc as bacc
import concourse.mybir as mybir
from concourse import bass_utils
from concourse.tile import TileContext

F32, BF16 = mybir.dt.float32, mybir.dt.bfloat16
AF = mybir.ActivationFunctionType
ALU = mybir.AluOpType
AX = mybir.AxisListType
BF = ml_dtypes.bfloat16

N, T, D, H = 256, 64, 1024, 1024
NCORES = 8
NL = N // NCORES          # 32 samples per core
HC = 8                    # 128-row chunks of D/H
H4 = 4 * H                # 4096 gate columns
QT = 16                   # steps per x quarter

_built = None


def _consts():
    # E16[k', 8k + ng] = (k' == k): expands wT rows onto the (k, ng) layout.
    e16 = np.zeros((16, 128), dtype=BF)
    for k in range(16):
        e16[k, 8 * k : 8 * k + 8] = 1
    # M128[p, 32g + m] = (m % 8 == p % 8) & (m // 8 == g): group-g selector.
    p = np.arange(128)[:, None]
    m = np.arange(32)[None, :]
    m128 = np.zeros((128, 128), dtype=BF)
    for g in range(4):
        m128[:, 32 * g : 32 * (g + 1)] = ((m % 8 == p % 8) & (m // 8 == g)).astype(BF)
    # Mdiag[32b + n, 32k + n'] = (n == n') / 32: diagonal extract + 1/sqrt(H)
    # scale, replicated over the 4 partition blocks of the column-tiled psc.
    md = np.zeros((128, 512), dtype=np.float32)
    n_ = np.arange(32)
    for b in range(4):
        for k in range(16):
            md[32 * b + n_, 32 * k + n_] = 1.0 / 32.0
    # S4[32b + n', n] = (n' == n): partition-block sum via PE.
    s4 = np.zeros((128, 32), dtype=np.float32)
    for b in range(4):
        s4[32 * b + n_, n_] = 1.0
    return e16, m128, md, s4


def _build_nc(t_steps=T):
    nc = bacc.Bacc(trn_type="TRN2", target_bir_lowering=False, debug=False)

    ap_xT = nc.dram_tensor("xT", [D, T * NL], BF16, kind="ExternalInput").ap()
    ap_Asc = nc.dram_tensor("Asc", [H, 512], BF16, kind="ExternalInput").ap()
    ap_Wx = nc.dram_tensor("Wx", [D, H4], BF16, kind="ExternalInput").ap()
    ap_Wh = nc.dram_tensor("Wh", [H, H4], BF16, kind="ExternalInput").ap()
    ap_Wattn = nc.dram_tensor("Wattn", [H, H4], BF16, kind="ExternalInput").ap()
    ap_bP = nc.dram_tensor("bP", [128, H4], BF16, kind="ExternalInput").ap()
    # h0T[p, 128r + 32qh + n] = h0[n, 512r + 128qh + p]; c0[32qh + n, 128r + p]
    ap_h0T = nc.dram_tensor("h0T", [128, 256], BF16, kind="ExternalInput").ap()
    ap_c0 = nc.dram_tensor("c0", [128, 256], F32, kind="ExternalInput").ap()
    # outT2[t, r, 32*qh + n, p] = h_t[n, 512r + 128qh + p]
    outT2 = nc.dram_tensor("outT2", [T, 2, 128, 128], BF16, kind="ExternalOutput").ap()

    e16_np, m128_np, md_np, s4_np = _consts()
    eye_d = nc.inline_tensor(np.eye(128, dtype=np.float32), "c_eye")
    eyeb_d = nc.inline_tensor(np.eye(128, dtype=BF), "c_eyeb")
    e16_d = nc.inline_tensor(e16_np, "c_e16")
    m128_d = nc.inline_tensor(m128_np, "c_m128")
    md_d = nc.inline_tensor(md_np, "c_mdiag")
    s4_d = nc.inline_tensor(s4_np, "c_s4")

    with TileContext(nc) as tc:
        with tc.tile_pool(name="pers", bufs=1) as pers:
            Wh_sb = pers.tile([128, HC * H4], BF16, tag="Wh")
            Asc_sb = pers.tile([128, HC * 512], BF16, tag="Asc")
            P_sb = pers.tile([128, 4 * H4], BF16, tag="P")
            xq = [pers.tile([128, HC * 512], BF16, tag=f"xq{b}", name=f"xq{b}")
                  for b in range(2)]
            Wxa = pers.tile([128, 4 * H4], BF16, tag="Wxa")
            cT = pers.tile([128, 256], F32, tag="cT")
            uThT = [pers.tile([128, 128], BF16, tag=f"uT{r}", name=f"uT{r}")
                    for r in range(2)]
            eye = pers.tile([128, 128], F32, tag="eye")
            eyeb = pers.tile([128, 128], BF16, tag="eyeb")
            E16 = pers.tile([16, 128], BF16, tag="E16")
            M128 = pers.tile([128, 128], BF16, tag="M128")
            Mdiag = pers.tile([128, 512], F32, tag="Mdiag")
            S4 = pers.tile([128, 32], F32, tag="S4")
            wsq = pers.tile([32, 32], BF16, tag="wsq")

            nc.sync.dma_start(eye[:], eye_d.ap()[:])
            nc.sync.dma_start(eyeb[:], eyeb_d.ap()[:])
            nc.sync.dma_start(E16[:], e16_d.ap()[:])
            nc.sync.dma_start(M128[:], m128_d.ap()[:])
            nc.sync.dma_start(Mdiag[:], md_d.ap()[:])
            nc.sync.dma_start(S4[:], s4_d.ap()[:])
            nc.gpsimd.memset(wsq[:], 0.0)
            for c in range(HC):
                nc.sync.dma_start(
                    Asc_sb[:, c * 512 : (c + 1) * 512],
                    ap_Asc[128 * c : 128 * (c + 1), :],
                )
            # step-0 stationary columns only (the bulk of quarter 0 follows
            # after phase B's weight traffic)
            for c in range(HC):
                nc.sync.dma_start(
                    xq[0][:, c * 512 : c * 512 + 32],
                    ap_xT[128 * c : 128 * (c + 1), 0:32],
                )

            # ---------------- phase B: P = A @ Wattn (+ b) ----------------
            with tc.tile_pool(name="phb", bufs=1) as phb, \
                 tc.tile_pool(name="psB", bufs=2, space="PSUM") as psB:
                bPsb = phb.tile([128, H4], BF16, tag="bPsb")
                nc.sync.dma_start(bPsb[:], ap_bP[:])
                # contiguous staging of the group-selected A columns so the
                # matmul stationary operand has a single free dim
                Ag = phb.tile([128, 4 * HC * 128], BF16, tag="Ag")
                for g in range(4):
                    for c in range(HC):
                        nc.vector.tensor_copy(
                            Ag[:, (g * HC + c) * 128 : (g * HC + c) * 128 + 128],
                            Asc_sb[:, c * 512 : (c + 1) * 512].rearrange(
                                "p (k n) -> p k n", k=16
                            )[:, :, 8 * g : 8 * (g + 1)],
                        )
                # Wattn eighths 0/1 first (phase B's critical input); the
                # other recurrence loads are paced 1-2 chunks per eighth-slot
                # so later Wattn eighths never queue behind 9 MB of weights
                extras = [("wh", 0), ("wh", 1), ("wxa", 0), ("wh", 2),
                          ("wxa", 1), ("wh", 3), ("wxa", 2), ("wh", 4),
                          ("wxa", 3), ("wh", 5), ("wh", 6), ("wh", 7)]

                def emit_extra(kind, c):
                    if kind == "wh":
                        nc.sync.dma_start(
                            Wh_sb[:, c * H4 : (c + 1) * H4],
                            ap_Wh[128 * c : 128 * (c + 1), :],
                        )
                    elif kind == "wxa":
                        nc.sync.dma_start(
                            Wxa[:, c * H4 : (c + 1) * H4],
                            ap_Wx[128 * c : 128 * (c + 1), :],
                        )
                    else:
                        nc.sync.dma_start(
                            xq[0][:, c * 512 + 32 : (c + 1) * 512],
                            ap_xT[128 * c : 128 * (c + 1), 32:512],
                        )

                wab_tiles = {}
                for e in range(2):
                    Wab = phb.tile([128, HC * 512], BF16, tag=f"wab{e % 3}",
                                   name=f"wab_{e}", bufs=1)
                    wab_tiles[e] = Wab
                    for c in range(HC):
                        nc.sync.dma_start(
                            Wab[:, c * 512 : (c + 1) * 512],
                            ap_Wattn[128 * c : 128 * (c + 1), 512 * e : 512 * (e + 1)],
                        )
                ex_i = 0
                for e in range(8):
                    if e < 2:
                        Wab = wab_tiles[e]
                    else:
                        Wab = phb.tile([128, HC * 512], BF16, tag=f"wab{e % 3}",
                                       name=f"wab_{e}", bufs=1)
                        for c in range(HC):
                            nc.sync.dma_start(
                                Wab[:, c * 512 : (c + 1) * 512],
                                ap_Wattn[128 * c : 128 * (c + 1), 512 * e : 512 * (e + 1)],
                            )
                    while ex_i < len(extras) and ex_i < max(0, e - 1) * 2:
                        emit_extra(*extras[ex_i])
                        ex_i += 1
                    for g in range(4):
                        psp = psB.tile([128, 512], F32, tag="psp",
                                       name=f"psp_{e}_{g}")
                        for c in range(HC):
                            nc.tensor.matmul(
                                psp[:],
                                Ag[:, (g * HC + c) * 128 : (g * HC + c) * 128 + 128],
                                Wab[:, c * 512 : (c + 1) * 512],
                                start=(c == 0),
                                stop=(c == HC - 1),
                            )
                        nc.vector.tensor_add(
                            P_sb[:, g * H4 + 512 * e : g * H4 + 512 * (e + 1)],
                            psp[:],
                            bPsb[:, 512 * e : 512 * (e + 1)],
                        )
                while ex_i < len(extras):
                    emit_extra(*extras[ex_i])
                    ex_i += 1
                for c in range(HC):
                    emit_extra("xq0bulk", c)

            # ---------------- h0 = c0 = mean_k(A): host-computed ----------------
            nc.sync.dma_start(uThT[0][:], ap_h0T[:, 0:128])
            nc.sync.dma_start(uThT[1][:], ap_h0T[:, 128:256])
            nc.sync.dma_start(cT[:], ap_c0[:])

            # ---------------------- recurrence ----------------------
            with tc.tile_pool(name="pers2", bufs=1) as pers2, \
                 tc.tile_pool(name="wrk", bufs=1) as wrk, \
                 tc.tile_pool(name="pstr", bufs=2, space="PSUM") as pstr, \
                 tc.tile_pool(name="pscp", bufs=1, space="PSUM") as pscp, \
                 tc.tile_pool(name="psm1", bufs=1, space="PSUM") as psm1, \
                 tc.tile_pool(name="psm3", bufs=1, space="PSUM") as psm3:
                Wxb = pers2.tile([128, 4 * H4], BF16, tag="Wxb")
                for c in range(4):
                    nc.sync.dma_start(
                        Wxb[:, c * H4 : (c + 1) * H4],
                        ap_Wx[128 * (c + 4) : 128 * (c + 5), :],
                    )
                # weights arrive with columns pre-permuted to (r, qh, j, p),
                # so each (c, r, qh) moving block is one contiguous 512-slice
                def wslice(W, c, r, qh):
                    base = c * H4 + (4 * r + qh) * 512
                    return W[:, base : base + 512]

                def wxslice(c, r, qh):
                    if c < 4:
                        return wslice(Wxa, c, r, qh)
                    return wslice(Wxb, c - 4, r, qh)

                def emit_xproj(t, strips):
                    qi, tq = t // QT, t % QT
                    xv = xq[qi % 2]
                    for r in range(2):
                        for c in range(HC):
                            stat = xv[:, c * 512 + 32 * tq : c * 512 + 32 * tq + 32]
                            for qh in range(4):
                                nc.tensor.matmul(
                                    strips[r][32 * qh : 32 * (qh + 1), :],
                                    stat,
                                    wxslice(c, r, qh),
                                    start=(c == 0),
                                    stop=False,
                                    skip_group_check=True,
                                    tile_position=(0, 32 * qh),
                                )

                strips = [pstr.tile([128, 512], F32, tag=f"strip{r}",
                                    name=f"strip{r}_0") for r in range(2)]
                emit_xproj(0, strips)

                for t in range(t_steps):
                    # prefetch next x quarter
                    if t % QT == 0 and (t // QT) + 1 < 4 and t + QT < t_steps:
                        qn = (t // QT) + 1
                        dst = xq[qn % 2]
                        for c in range(HC):
                            nc.sync.dma_start(
                                dst[:, c * 512 : (c + 1) * 512],
                                ap_xT[128 * c : 128 * (c + 1),
                                      512 * qn : 512 * (qn + 1)],
                            )

                    # -- scores: 4 column-tiled chains, 2-deep accumulation
                    psc = pscp.tile([128, 512], F32, tag="psc", name=f"psc_{t}")
                    for c in range(HC):
                        r_c, qh_c = c // 4, c % 4
                        nc.tensor.matmul(
                            psc[32 * qh_c : 32 * (qh_c + 1), :],
                            uThT[r_c][:, 32 * qh_c : 32 * (qh_c + 1)],
                            Asc_sb[:, c * 512 : (c + 1) * 512],
                            start=(c < 4),
                            stop=(c >= 4),
                            skip_group_check=True,
                            tile_position=(0, 32 * qh_c),
                        )
                    scm = wrk.tile([128, 512], F32, tag="scm", name=f"scm_{t}")
                    nc.vector.tensor_mul(scm[:], psc[:], Mdiag[:])
                    scpart = wrk.tile([128, 16], F32, tag="scp", name=f"scp_{t}")
                    nc.vector.tensor_reduce(
                        scpart[:],
                        scm[:].rearrange("p (k n) -> p k n", k=16),
                        axis=AX.X,
                        op=ALU.add,
                    )

                    # -- gates: h @ Wh (first half; blocksum MM slots after)
                    for r in range(2):
                        for c in range(0, 4):
                            stat = uThT[c // 4][:, 32 * (c % 4) : 32 * (c % 4) + 32]
                            for qh in range(4):
                                nc.tensor.matmul(
                                    strips[r][32 * qh : 32 * (qh + 1), :],
                                    stat,
                                    wslice(Wh_sb, c, r, qh),
                                    start=False,
                                    stop=False,
                                    skip_group_check=True,
                                    tile_position=(0, 32 * qh),
                                )
                    # blocksum: scores[n, k] = sum_b scpart[32b + n, k]
                    # (scoresP + pwx share one PSUM bank)
                    psmall = psm1.tile([32, 16], F32, tag="psmall",
                                       name=f"psmall_{t}")
                    scoresP = psmall[:]
                    nc.tensor.matmul(scoresP, S4[:], scpart[:],
                                     start=True, stop=True)
                    for r in range(2):
                        for c in range(4, HC):
                            stat = uThT[c // 4][:, 32 * (c % 4) : 32 * (c % 4) + 32]
                            for qh in range(4):
                                nc.tensor.matmul(
                                    strips[r][32 * qh : 32 * (qh + 1), :],
                                    stat,
                                    wslice(Wh_sb, c, r, qh),
                                    start=False,
                                    stop=False,
                                    skip_group_check=True,
                                    tile_position=(0, 32 * qh),
                                )

                    # -- softmax (exp via sigmoid: no ACT table swap; scores
                    # are bounded |s| < ~4 so no max-subtraction needed)
                    sig = wrk.tile([32, 16], F32, tag="sig", name=f"sig_{t}")
                    nc.scalar.activation(sig[:], scoresP, AF.Sigmoid)
                    om = wrk.tile([32, 16], F32, tag="om", name=f"om_{t}")
                    nc.vector.tensor_scalar(
                        om[:], sig[:], -1.0, 1.0, op0=ALU.mult, op1=ALU.add
                    )
                    rom = wrk.tile([32, 16], F32, tag="rom", name=f"rom_{t}")
                    nc.vector.reciprocal(rom[:], om[:])
                    ex = wrk.tile([32, 16], F32, tag="ex", name=f"ex_{t}")
                    nc.vector.tensor_mul(ex[:], sig[:], rom[:])
                    esum = wrk.tile([32, 1], F32, tag="esum", name=f"esum_{t}")
                    nc.vector.tensor_reduce(esum[:], ex[:], axis=AX.X, op=ALU.add)
                    rcp = wrk.tile([32, 1], F32, tag="rcp", name=f"rcp_{t}")
                    nc.vector.reciprocal(rcp[:], esum[:])
                    nc.vector.tensor_scalar_mul(wsq[:, 0:16], ex[:], rcp[:])
                    wT = wrk.tile([32, 32], BF16, tag="wT", name=f"wT_{t}")
                    nc.vector.transpose(wT[:], wsq[:])
                    wTb = wT[0:16, :]

                    # -- expand w to the (k, ng) block layout
                    pwxt = psm3.tile([128, 32], F32, tag="pwx", name=f"pwx_{t}")
                    pwx = pwxt[:]
                    nc.tensor.matmul(pwx, E16[:], wTb, start=True, stop=True)
                    masked = wrk.tile([128, 128], BF16, tag="masked",
                                      name=f"masked_{t}")
                    for g in range(4):
                        nc.vector.tensor_mul(
                            masked[:, 32 * g : 32 * (g + 1)],
                            pwx[:],
                            M128[:, 32 * g : 32 * (g + 1)],
                        )

                    # -- gates: attention term sum_k w_k P_k (+ b)
                    for r in range(2):
                        for g in range(4):
                            stat = masked[:, 32 * g : 32 * (g + 1)]
                            for qh in range(4):
                                nc.tensor.matmul(
                                    strips[r][32 * qh : 32 * (qh + 1), :],
                                    stat,
                                    P_sb[:, g * H4 + (4 * r + qh) * 512 :
                                         g * H4 + (4 * r + qh) * 512 + 512],
                                    start=False,
                                    stop=(g == 3),
                                    skip_group_check=True,
                                    tile_position=(0, 32 * qh),
                                )

                    # -- next step's xproj fills the PE while the tail runs
                    if t + 1 < t_steps:
                        nstrips = [pstr.tile([128, 512], F32, tag=f"strip{r}",
                                             name=f"strip{r}_{t + 1}")
                                   for r in range(2)]
                        emit_xproj(t + 1, nstrips)
                    else:
                        nstrips = None

                    # -- activations + cell update in strip space
                    # (ACT queue is in-order: issue both halves' sigmoids/tanh
                    # before the cell chains so tanh(c) of half 0 doesn't
                    # block sigmoid of half 1)
                    acts = []
                    for r in range(2):
                        act = wrk.tile([128, 512], F32, tag=f"act{r}",
                                       name=f"act{r}_{t}")
                        nc.scalar.activation(act[:, 0:384], strips[r][:, 0:384],
                                             AF.Sigmoid)
                        nc.scalar.activation(act[:, 384:512], strips[r][:, 384:512],
                                             AF.Tanh)
                        acts.append(act)
                    pTT = psm1.tile([128, 256], BF16, tag="pTT", name=f"pTT_{t}")
                    for r in range(2):
                        act = acts[r]
                        cv = cT[:, 128 * r : 128 * (r + 1)]
                        ig = wrk.tile([128, 128], F32, tag=f"ig{r}",
                                      name=f"ig{r}_{t}")
                        nc.vector.tensor_mul(ig[:], act[:, 0:128], act[:, 384:512])
                        fc = wrk.tile([128, 128], F32, tag=f"fc{r}",
                                      name=f"fc{r}_{t}")
                        nc.vector.tensor_mul(fc[:], act[:, 128:256], cv)
                        nc.vector.tensor_add(cv, ig[:], fc[:])
                        tch = wrk.tile([128, 128], F32, tag=f"tch{r}",
                                       name=f"tch{r}_{t}")
                        nc.scalar.activation(tch[:], cv, AF.Tanh)
                        hb = wrk.tile([128, 128], BF16, tag=f"hb{r}",
                                      name=f"hb{r}_{t}")
                        nc.vector.tensor_mul(hb[:], act[:, 256:384], tch[:])
                        nc.sync.dma_start(outT2[t, r], hb[:])
                        pT = pTT[:, 128 * r : 128 * (r + 1)]
                        nc.tensor.matmul(pT, hb[:], eyeb[:],
                                         is_transpose=True, start=True, stop=True)
                        nc.vector.tensor_copy(uThT[r][:], pT)

                    strips = nstrips
    nc.compile()
    return nc


def _prep_shards(inputs):
    x = np.asarray(inputs["x"], np.float32)
    A = np.asarray(inputs["A"], np.float32)
    Wx = np.asarray(inputs["Wx"], np.float32)
    Wh = np.asarray(inputs["Wh"], np.float32)
    Wattn = np.asarray(inputs["Wattn"], np.float32)
    b = np.asarray(inputs["b"], np.float32)

    # permute gate columns (j, r, qh, p) -> (r, qh, j, p) so device moving
    # blocks are contiguous
    def _perm(W):
        return np.ascontiguousarray(
            W.reshape(-1, 4, 2, 4, 128).transpose(0, 2, 3, 1, 4).reshape(-1, H4)
        )

    Wx_bf = _perm(Wx).astype(BF)
    Wh_bf = _perm(Wh).astype(BF)
    Wa_bf = _perm(Wattn).astype(BF)
    bp = _perm(b.reshape(1, H4)).reshape(H4)
    bP = np.ascontiguousarray(np.tile(bp.astype(BF)[None, :], (128, 1)))

    in_maps = []
    for i in range(NCORES):
        ns = slice(NL * i, NL * (i + 1))
        xT = x[ns].transpose(2, 1, 0).reshape(D, T * NL)
        Asc = A[ns].reshape(NL, H, 16).transpose(1, 2, 0).reshape(H, 512)
        h0 = A[ns].reshape(NL, H, 16).mean(axis=2)          # [32, 1024]
        h05 = h0.reshape(NL, 2, 4, 128)                     # [n, r, qh, p]
        h0T = h05.transpose(3, 1, 2, 0).reshape(128, 256)   # [p, (r qh n)]
        c0 = h05.transpose(2, 0, 1, 3).reshape(128, 256)    # [(qh n), (r p)]
        in_maps.append(
            {
                "xT": np.ascontiguousarray(xT.astype(BF)),
                "Asc": np.ascontiguousarray(Asc.astype(BF)),
                "Wx": Wx_bf,
                "Wh": Wh_bf,
                "Wattn": Wa_bf,
                "bP": bP,
                "h0T": np.ascontiguousarray(h0T.astype(BF)),
                "c0": np.ascontiguousarray(c0.astype(np.float32)),
            }
        )
    return in_maps


def _get_nc():
    global _built
    if _built is None:
        _built = _build_nc()
    return _built


def _run(inputs, **kwargs):
    nc = _get_nc()
    in_maps = _prep_shards(inputs)
    res = bass_utils.run_bass_kernel_spmd(
        nc, in_maps, core_ids=list(range(NCORES)), **kwargs
    )
    out = np.empty((N, T, H), np.float32)
    for i in range(NCORES):
        o = np.asarray(res.results[i]["outT2"], dtype=np.float32)
        out[NL * i : NL * (i + 1)] = (
            o.reshape(T, 2, 4, 32, 128).transpose(3, 0, 1, 2, 4).reshape(NL, T, H)
        )
    return out, res


def kernel(**inputs):
    out, _ = _run(inputs)
    return out


# revision 15
# speedup vs baseline: 1.1587x; 1.1587x over previous
"""Trainium2 Bass kernel for nn_CaptioningRNN (attention LSTM over T=64).

Data-parallel over the batch: N=256 samples split across 8 NeuronCores
(32 samples/core), weights replicated, no collectives.

Per-core algorithm (v3):
  - No xproj prepass: x @ Wx is accumulated directly into each step's gate
    strips on the TensorEngine (x-chunk stationary [128,32], Wx moving,
    4-way column tiling), emitted one step ahead so it executes during the
    previous step's vector/scalar tail. This removes the xps DRAM scratch
    round-trip entirely.
  - Strips are laid out [32*qh + n, (j, p)]: the 4 PE column-tile quadrants
    map to 128-column blocks (qh) of the hidden dim, NOT the gate index, so
    i/f/o/g for one (n, h') live on one partition and the LSTM cell math
    runs directly in strip space (no [128,512] transposes).
  - P[n,k,:] = A[n,:,k] @ Wattn precomputed once; the bias b is folded into
    P (softmax weights sum to 1), so gates = xWx + hWh + sum_k w_k P_k
    includes +b exactly.
  - scores via 4 column-tiled accumulation chains into one [128,512] PSUM,
    diag-masked and reduced on DVE, block-summed with a tiny PE matmul.
  - softmax exp computed as sigmoid(s-m)/(1-sigmoid(s-m)) so the scalar
    engine never swaps activation tables (Exp <-> Sigmoid/Tanh reload costs
    ~2.6us/step otherwise).
  - h is produced per 512-block as [s, p] f32, DMA'd straight to the output,
    and PE-transposed ([128,128]) to hT bf16 for the next step's stationary.
  - x streamed in quarters (16 steps each) to fit SBUF next to Wx+Wh+P.
"""

import numpy as np
import ml_dtypes

import concourse.bacc as bacc
import concourse.mybir as mybir
from concourse import bass_utils
from concourse.tile import TileContext

F32, BF16 = mybir.dt.float32, mybir.dt.bfloat16
AF = mybir.ActivationFunctionType
ALU = mybir.AluOpType
AX = mybir.AxisListType
BF = ml_dtypes.bfloat16

N, T, D, H = 256, 64, 1024, 1024
NCORES = 8
NL = N // NCORES          # 32 samples per core
HC = 8                    # 128-row chunks of D/H
H4 = 4 * H                # 4096 gate columns
QT = 16                   # steps per x quarter

_built = None


def _consts():
    # E16[k', 8k + ng] = (k' == k): expands wT rows onto the (k, ng) layout.
    e16 = np.zeros((16, 128), dtype=BF)
    for k in range(16):
        e16[k, 8 * k : 8 * k + 8] = 1
    # M128[p, 32g + m] = (m % 8 == p % 8) & (m // 8 == g): group-g selector.
    p = np.arange(128)[:, None]
    m = np.arange(32)[None, :]
    m128 = np.zeros((128, 128), dtype=BF)
    for g in range(4):
        m128[:, 32 * g : 32 * (g + 1)] = ((m % 8 == p % 8) & (m // 8 == g)).astype(BF)
    # Mdiag[32b + n, 32k + n'] = (n == n') / 32: diagonal extract + 1/sqrt(H)
    # scale, replicated over the 4 partition blocks of the column-tiled psc.
    md = np.zeros((128, 512), dtype=np.float32)
    n_ = np.arange(32)
    for b in range(4):
        for k in range(16):
            md[32 * b + n_, 32 * k + n_] = 1.0 / 32.0
    # S4[32b + n', n] = (n' == n): partition-block sum via PE.
    s4 = np.zeros((128, 32), dtype=np.float32)
    for b in range(4):
        s4[32 * b + n_, n_] = 1.0
    return e16, m128, md, s4


def _build_nc(t_steps=T):
    nc = bacc.Bacc(trn_type="TRN2", target_bir_lowering=False, debug=False)

    ap_xT = nc.dram_tensor("xT", [D, T * NL], BF16, kind="ExternalInput").ap()
    ap_Asc = nc.dram_tensor("Asc", [H, 512], BF16, kind="ExternalInput").ap()
    ap_Wx = nc.dram_tensor("Wx", [D, H4], BF16, kind="ExternalInput").ap()
    ap_Wh = nc.dram_tensor("Wh", [H, H4], BF16, kind="ExternalInput").ap()
    ap_Wattn = nc.dram_tensor("Wattn", [H, H4], BF16, kind="ExternalInput").ap()
    ap_bP = nc.dram_tensor("bP", [128, H4], BF16, kind="ExternalInput").ap()
    # h0T[p, 128r + 32qh + n] = h0[n, 512r + 128qh + p]; c0[32qh + n, 128r + p]
    ap_h0T = nc.dram_tensor("h0T", [128, 256], BF16, kind="ExternalInput").ap()
    ap_c0 = nc.dram_tensor("c0", [128, 256], F32, kind="ExternalInput").ap()
    # outT2[t, r, 32*qh + n, p] = h_t[n, 512r + 128qh + p]
    outT2 = nc.dram_tensor("outT2", [T, 2, 128, 128], BF16, kind="ExternalOutput").ap()

    e16_np, m128_np, md_np, s4_np = _consts()
    eye_d = nc.inline_tensor(np.eye(128, dtype=np.float32), "c_eye")
    eyeb_d = nc.inline_tensor(np.eye(128, dtype=BF), "c_eyeb")
    e16_d = nc.inline_tensor(e16_np, "c_e16")
    m128_d = nc.inline_tensor(m128_np, "c_m128")
    md_d = nc.inline_tensor(md_np, "c_mdiag")
    s4_d = nc.inline_tensor(s4_np, "c_s4")

    with TileContext(nc) as tc:
        with tc.tile_pool(name="pers", bufs=1) as pers:
            Wh_sb = pers.tile([128, HC * H4], BF16, tag="Wh")
            Asc_sb = pers.tile([128, HC * 512], BF16, tag="Asc")
            P_sb = pers.tile([128, 4 * H4], BF16, tag="P")
            xq = [pers.tile([128, HC * 512], BF16, tag=f"xq{b}", name=f"xq{b}")
                  for b in range(2)]
            cT = pers.tile([128, 256], F32, tag="cT")
            uThT = [pers.tile([128, 128], BF16, tag=f"uT{r}", name=f"uT{r}")
                    for r in range(2)]
            eye = pers.tile([128, 128], F32, tag="eye")
            eyeb = pers.tile([128, 128], BF16, tag="eyeb")
            E16 = pers.tile([16, 128], BF16, tag="E16")
            M128 = pers.tile([128, 128], BF16, tag="M128")
            Mdiag = pers.tile([128, 512], F32, tag="Mdiag")
            S4 = pers.tile([128, 32], F32, tag="S4")
            wsq = pers.tile([32, 32], BF16, tag="wsq")

            nc.sync.dma_start(eye[:], eye_d.ap()[:])
            nc.sync.dma_start(eyeb[:], eyeb_d.ap()[:])
            nc.sync.dma_start(E16[:], e16_d.ap()[:])
            nc.sync.dma_start(M128[:], m128_d.ap()[:])
            nc.sync.dma_start(Mdiag[:], md_d.ap()[:])
            nc.sync.dma_start(S4[:], s4_d.ap()[:])
            nc.gpsimd.memset(wsq[:], 0.0)
            for c in range(HC):
                nc.sync.dma_start(
                    Asc_sb[:, c * 512 : (c + 1) * 512],
                    ap_Asc[128 * c : 128 * (c + 1), :],
                )

            # ---------------- phase B: P = A @ Wattn (+ b) ----------------
            with tc.tile_pool(name="phb", bufs=1) as phb, \
                 tc.tile_pool(name="psB", bufs=2, space="PSUM") as psB:
                bPsb = phb.tile([128, H4], BF16, tag="bPsb")
                nc.sync.dma_start(bPsb[:], ap_bP[:])
                # contiguous staging of the group-selected A columns so the
                # matmul stationary operand has a single free dim
                Ag = phb.tile([128, 4 * HC * 128], BF16, tag="Ag")
                for g in range(4):
                    for c in range(HC):
                        nc.vector.tensor_copy(
                            Ag[:, (g * HC + c) * 128 : (g * HC + c) * 128 + 128],
                            Asc_sb[:, c * 512 : (c + 1) * 512].rearrange(
                                "p (k n) -> p k n", k=16
                            )[:, :, 8 * g : 8 * (g + 1)],
                        )
                # Wattn eighths 0/1 first (phase B's critical input); the
                # other recurrence loads are paced 1-2 chunks per eighth-slot
                # so later Wattn eighths never queue behind 9 MB of weights
                extras = (
                    [("wh", c) for c in range(HC)] + [("xq", c) for c in range(HC)]
                )

                def emit_extra(kind, c):
                    if kind == "wh":
                        nc.sync.dma_start(
                            Wh_sb[:, c * H4 : (c + 1) * H4],
                            ap_Wh[128 * c : 128 * (c + 1), :],
                        )
                    else:
                        nc.sync.dma_start(
                            xq[0][:, c * 512 : (c + 1) * 512],
                            ap_xT[128 * c : 128 * (c + 1), 0:512],
                        )

                wab_tiles = {}
                for e in range(2):
                    Wab = phb.tile([128, HC * 512], BF16, tag=f"wab{e % 4}",
                                   name=f"wab_{e}", bufs=1)
                    wab_tiles[e] = Wab
                    for c in range(HC):
                        nc.sync.dma_start(
                            Wab[:, c * 512 : (c + 1) * 512],
                            ap_Wattn[128 * c : 128 * (c + 1), 512 * e : 512 * (e + 1)],
                        )
                ex_i = 0
                for e in range(8):
                    if e < 2:
                        Wab = wab_tiles[e]
                    else:
                        Wab = phb.tile([128, HC * 512], BF16, tag=f"wab{e % 4}",
                                       name=f"wab_{e}", bufs=1)
                        for c in range(HC):
                            nc.sync.dma_start(
                                Wab[:, c * 512 : (c + 1) * 512],
                                ap_Wattn[128 * c : 128 * (c + 1), 512 * e : 512 * (e + 1)],
                            )
                    while ex_i < len(extras) and ex_i < (e + 1) * 3 - 2:
                        emit_extra(*extras[ex_i])
                        ex_i += 1
                    for g in range(4):
                        psp = psB.tile([128, 512], F32, tag="psp",
                                       name=f"psp_{e}_{g}")
                        for c in range(HC):
                            nc.tensor.matmul(
                                psp[:],
                                Ag[:, (g * HC + c) * 128 : (g * HC + c) * 128 + 128],
                                Wab[:, c * 512 : (c + 1) * 512],
                                start=(c == 0),
                                stop=(c == HC - 1),
                            )
                        nc.vector.tensor_add(
                            P_sb[:, g * H4 + 512 * e : g * H4 + 512 * (e + 1)],
                            psp[:],
                            bPsb[:, 512 * e : 512 * (e + 1)],
                        )
                while ex_i < len(extras):
                    emit_extra(*extras[ex_i])
                    ex_i += 1

            # ---------------- h0 = c0 = mean_k(A): host-computed ----------------
            nc.sync.dma_start(uThT[0][:], ap_h0T[:, 0:128])
            nc.sync.dma_start(uThT[1][:], ap_h0T[:, 128:256])
            nc.sync.dma_start(cT[:], ap_c0[:])

            # ---------------------- recurrence ----------------------
            with tc.tile_pool(name="pers2", bufs=1) as pers2, \
                 tc.tile_pool(name="wrk", bufs=1) as wrk, \
                 tc.tile_pool(name="pstr", bufs=2, space="PSUM") as pstr, \
                 tc.tile_pool(name="pscp", bufs=1, space="PSUM") as pscp, \
                 tc.tile_pool(name="psm1", bufs=1, space="PSUM") as psm1, \
                 tc.tile_pool(name="psm3", bufs=1, space="PSUM") as psm3:
                Wx_sb = pers2.tile([128, HC * H4], BF16, tag="Wx")
                for c in range(HC):
                    nc.sync.dma_start(
                        Wx_sb[:, c * H4 : (c + 1) * H4],
                        ap_Wx[128 * c : 128 * (c + 1), :],
                    )
                # weights arrive with columns pre-permuted to (r, qh, j, p),
                # so each (c, r, qh) moving block is one contiguous 512-slice
                def wslice(W, c, r, qh):
                    base = c * H4 + (4 * r + qh) * 512
                    return W[:, base : base + 512]

                def emit_xproj(t, strips):
                    qi, tq = t // QT, t % QT
                    xv = xq[qi % 2]
                    for r in range(2):
                        for c in range(HC):
                            stat = xv[:, c * 512 + 32 * tq : c * 512 + 32 * tq + 32]
                            for qh in range(4):
                                nc.tensor.matmul(
                                    strips[r][32 * qh : 32 * (qh + 1), :],
                                    stat,
                                    wslice(Wx_sb, c, r, qh),
                                    start=(c == 0),
                                    stop=False,
                                    skip_group_check=True,
                                    tile_position=(0, 32 * qh),
                                )

                strips = [pstr.tile([128, 512], F32, tag=f"strip{r}",
                                    name=f"strip{r}_0") for r in range(2)]
                emit_xproj(0, strips)

                for t in range(t_steps):
                    # prefetch next x quarter
                    if t % QT == 0 and (t // QT) + 1 < 4 and t + QT < t_steps:
                        qn = (t // QT) + 1
                        dst = xq[qn % 2]
                        for c in range(HC):
                            nc.sync.dma_start(
                                dst[:, c * 512 : (c + 1) * 512],
                                ap_xT[128 * c : 128 * (c + 1),
                                      512 * qn : 512 * (qn + 1)],
                            )

                    # -- scores: 4 column-tiled chains, 2-deep accumulation
                    psc = pscp.tile([128, 512], F32, tag="psc", name=f"psc_{t}")
                    for c in range(HC):
                        r_c, qh_c = c // 4, c % 4
                        nc.tensor.matmul(
                            psc[32 * qh_c : 32 * (qh_c + 1), :],
                            uThT[r_c][:, 32 * qh_c : 32 * (qh_c + 1)],
                            Asc_sb[:, c * 512 : (c + 1) * 512],
                            start=(c < 4),
                            stop=(c >= 4),
                            skip_group_check=True,
                            tile_position=(0, 32 * qh_c),
                        )
                    scm = wrk.tile([128, 512], F32, tag="scm", name=f"scm_{t}")
                    nc.vector.tensor_mul(scm[:], psc[:], Mdiag[:])
                    scpart = wrk.tile([128, 16], F32, tag="scp", name=f"scp_{t}")
                    nc.vector.tensor_reduce(
                        scpart[:],
                        scm[:].rearrange("p (k n) -> p k n", k=16),
                        axis=AX.X,
                        op=ALU.add,
                    )

                    # -- gates: h @ Wh (first half; blocksum MM slots after)
                    for r in range(2):
                        for c in range(0, 4):
                            stat = uThT[c // 4][:, 32 * (c % 4) : 32 * (c % 4) + 32]
                            for qh in range(4):
                                nc.tensor.matmul(
                                    strips[r][32 * qh : 32 * (qh + 1), :],
                                    stat,
                                    wslice(Wh_sb, c, r, qh),
                                    start=False,
                                    stop=False,
                                    skip_group_check=True,
                                    tile_position=(0, 32 * qh),
                                )
                    # blocksum: scores[n, k] = sum_b scpart[32b + n, k]
                    # (scoresP + pwx share one PSUM bank)
                    psmall = psm1.tile([32, 16], F32, tag="psmall",
                                       name=f"psmall_{t}")
                    scoresP = psmall[:]
                    nc.tensor.matmul(scoresP, S4[:], scpart[:],
                                     start=True, stop=True)
                    for r in range(2):
                        for c in range(4, HC):
                            stat = uThT[c // 4][:, 32 * (c % 4) : 32 * (c % 4) + 32]
                            for qh in range(4):
                                nc.tensor.matmul(
                                    strips[r][32 * qh : 32 * (qh + 1), :],
                                    stat,
                                    wslice(Wh_sb, c, r, qh),
                                    start=False,
                                    stop=False,
                                    skip_group_check=True,
                                    tile_position=(0, 32 * qh),
                                )

                    # -- softmax (exp via sigmoid: no ACT table swap; scores
                    # are bounded |s| < ~4 so no max-subtraction needed)
                    sig = wrk.tile([32, 16], F32, tag="sig", name=f"sig_{t}")
                    nc.scalar.activation(sig[:], scoresP, AF.Sigmoid)
                    om = wrk.tile([32, 16], F32, tag="om", name=f"om_{t}")
                    nc.vector.tensor_scalar(
                        om[:], sig[:], -1.0, 1.0, op0=ALU.mult, op1=ALU.add
                    )
                    rom = wrk.tile([32, 16], F32, tag="rom", name=f"rom_{t}")
                    nc.vector.reciprocal(rom[:], om[:])
                    ex = wrk.tile([32, 16], F32, tag="ex", name=f"ex_{t}")
                    nc.vector.tensor_mul(ex[:], sig[:], rom[:])
                    esum = wrk.tile([32, 1], F32, tag="esum", name=f"esum_{t}")
                    nc.vector.tensor_reduce(esum[:], ex[:], axis=AX.X, op=ALU.add)
                    rcp = wrk.tile([32, 1], F32, tag="rcp", name=f"rcp_{t}")
                    nc.vector.reciprocal(rcp[:], esum[:])
                    nc.vector.tensor_scalar_mul(wsq[:, 0:16], ex[:], rcp[:])
                    wT = wrk.tile([32, 32], BF16, tag="wT", name=f"wT_{t}")
                    nc.vector.transpose(wT[:], wsq[:])
                    wTb = wT[0:16, :]

                    # -- expand w to the (k, ng) block layout
                    pwxt = psm3.tile([128, 32], F32, tag="pwx", name=f"pwx_{t}")
                    pwx = pwxt[:]
                    nc.tensor.matmul(pwx, E16[:], wTb, start=True, stop=True)
                    masked = wrk.tile([128, 128], BF16, tag="masked",
                                      name=f"masked_{t}")
                    for g in range(4):
                        nc.vector.tensor_mul(
                            masked[:, 32 * g : 32 * (g + 1)],
                            pwx[:],
                            M128[:, 32 * g : 32 * (g + 1)],
                        )

                    # -- gates: attention term sum_k w_k P_k (+ b)
                    for r in range(2):
                        for g in range(4):
                            stat = masked[:, 32 * g : 32 * (g + 1)]
                            for qh in range(4):
                                nc.tensor.matmul(
                                    strips[r][32 * qh : 32 * (qh + 1), :],
                                    stat,
                                    P_sb[:, g * H4 + (4 * r + qh) * 512 :
                                         g * H4 + (4 * r + qh) * 512 + 512],
                                    start=False,
                                    stop=(g == 3),
                                    skip_group_check=True,
                                    tile_position=(0, 32 * qh),
                                )

                    # -- next step's xproj fills the PE while the tail runs
                    if t + 1 < t_steps:
                        nstrips = [pstr.tile([128, 512], F32, tag=f"strip{r}",
                                             name=f"strip{r}_{t + 1}")
                                   for r in range(2)]
                        emit_xproj(t + 1, nstrips)
                    else:
                        nstrips = None

                    # -- activations + cell update in strip space
                    # (ACT queue is in-order: issue both halves' sigmoids/tanh
                    # before the cell chains so tanh(c) of half 0 doesn't
                    # block sigmoid of half 1)
                    acts = []
                    for r in range(2):
                        act = wrk.tile([128, 512], F32, tag=f"act{r}",
                                       name=f"act{r}_{t}")
                        nc.scalar.activation(act[:, 0:384], strips[r][:, 0:384],
                                             AF.Sigmoid)
                        nc.scalar.activation(act[:, 384:512], strips[r][:, 384:512],
                                             AF.Tanh)
                        acts.append(act)
                    pTT = psm1.tile([128, 256], BF16, tag="pTT", name=f"pTT_{t}")
                    for r in range(2):
                        act = acts[r]
                        cv = cT[:, 128 * r : 128 * (r + 1)]
                        ig = wrk.tile([128, 128], F32, tag=f"ig{r}",
                                      name=f"ig{r}_{t}")
                        nc.vector.tensor_mul(ig[:], act[:, 0:128], act[:, 384:512])
                        fc = wrk.tile([128, 128], F32, tag=f"fc{r}",
                                      name=f"fc{r}_{t}")
                        nc.vector.tensor_mul(fc[:], act[:, 128:256], cv)
                        nc.vector.tensor_add(cv, ig[:], fc[:])
                        tch = wrk.tile([128, 128], F32, tag=f"tch{r}",
                                       name=f"tch{r}_{t}")
                        nc.scalar.activation(tch[:], cv, AF.Tanh)
                        hb = wrk.tile([128, 128], BF16, tag=f"hb{r}",
                                      name=f"hb{r}_{t}")
                        nc.vector.tensor_mul(hb[:], act[:, 256:384], tch[:])
                        nc.sync.dma_start(outT2[t, r], hb[:])
                        pT = pTT[:, 128 * r : 128 * (r + 1)]
                        nc.tensor.matmul(pT, hb[:], eyeb[:],
                                         is_transpose=True, start=True, stop=True)
                        nc.vector.tensor_copy(uThT[r][:], pT)

                    strips = nstrips
    nc.compile()
    return nc


def _prep_shards(inputs):
    x = np.asarray(inputs["x"], np.float32)
    A = np.asarray(inputs["A"], np.float32)
    Wx = np.asarray(inputs["Wx"], np.float32)
    Wh = np.asarray(inputs["Wh"], np.float32)
    Wattn = np.asarray(inputs["Wattn"], np.float32)
    b = np.asarray(inputs["b"], np.float32)

    # permute gate columns (j, r, qh, p) -> (r, qh, j, p) so device moving
    # blocks are contiguous
    def _perm(W):
        return np.ascontiguousarray(
            W.reshape(-1, 4, 2, 4, 128).transpose(0, 2, 3, 1, 4).reshape(-1, H4)
        )

    Wx_bf = _perm(Wx).astype(BF)
    Wh_bf = _perm(Wh).astype(BF)
    Wa_bf = _perm(Wattn).astype(BF)
    bp = _perm(b.reshape(1, H4)).reshape(H4)
    bP = np.ascontiguousarray(np.tile(bp.astype(BF)[None, :], (128, 1)))

    in_maps = []
    for i in range(NCORES):
        ns = slice(NL * i, NL * (i + 1))
        xT = x[ns].transpose(2, 1, 0).reshape(D, T * NL)
        Asc = A[ns].reshape(NL, H, 16).transpose(1, 2, 0).reshape(H, 512)
        h0 = A[ns].reshape(NL, H, 16).mean(axis=2)          # [32, 1024]
        h05 = h0.reshape(NL, 2, 4, 128)                     # [n, r, qh, p]
        h0T = h05.transpose(3, 1, 2, 0).reshape(128, 256)   # [p, (r qh n)]
        c0 = h05.transpose(2, 0, 1, 3).reshape(128, 256)    # [(qh n), (r p)]
        in_maps.append(
            {
                "xT": np.ascontiguousarray(xT.astype(BF)),
                "Asc": np.ascontiguousarray(Asc.astype(BF)),
                "Wx": Wx_bf,
                "Wh": Wh_bf,
                "Wattn": Wa_bf,
                "bP": bP,
                "h0T": np.ascontiguousarray(h0T.astype(BF)),
                "c0": np.ascontiguousarray(c0.astype(np.float32)),
            }
        )
    return in_maps


def _get_nc():
    global _built
    if _built is None:
        _built = _build_nc()
    return _built


def _run(inputs, **kwargs):
    nc = _get_nc()
    in_maps = _prep_shards(inputs)
    res = bass_utils.run_bass_kernel_spmd(
        nc, in_maps, core_ids=list(range(NCORES)), **kwargs
    )
    out = np.empty((N, T, H), np.float32)
    for i in range(NCORES):
        o = np.asarray(res.results[i]["outT2"], dtype=np.float32)
        out[NL * i : NL * (i + 1)] = (
            o.reshape(T, 2, 4, 32, 128).transpose(3, 0, 1, 2, 4).reshape(NL, T, H)
        )
    return out, res


def kernel(**inputs):
    out, _ = _run(inputs)
    return out
